# revision 9
# baseline (speedup 1.0000x reference)
"""LSTM-cell scan (masked Encoder) on 8 Trainium2 NeuronCores.

Problem: sequence [B=256, A=128, T=50, I=16] f32 through a single
LSTMCell (H=128) scanned over T; returns final (h, c), each [B, A, H].
The mask input is all-ones (per the problem spec), so the mask blend is
an identity and is skipped on-device (a host-side check falls back to a
numpy implementation if a non-trivial mask ever shows up).

Sharding: data-parallel over B across 8 cores (32 batch rows -> 4096
independent sequences per core). Params replicated.

Per-core kernel layout: hidden dim H=128 lives on SBUF partitions, the
4096 sequences on the free dim, split into 8 sub-tiles of 512 (PSUM bank
width). Per timestep and sub-tile, each gate g gets
    psum_g = W_ih_g^T.T @ x_t  (+)  W_hh_g^T.T @ h
as two accumulating fp32r matmuls, then ACT applies sigmoid/tanh with the
per-partition bias operand, and DVE/Pool do the cell update. x_t arrives
batch-major from DRAM and is transposed on-chip via the PE (8 timesteps
per [128,128] transpose). States h (fp32r, feeds matmul) and c (fp32)
persist in SBUF across all 50 steps; only final h/c are written out.
"""

import os
from contextlib import ExitStack

import numpy as np

N_CORES = 8
B, A, T, I, H = 256, 128, 50, 16, 128
NB = B // N_CORES          # batch rows per core
N = NB * A                 # sequences per core (4096)
NSUB = 8                   # N sub-tiles per core
NS = N // NSUB             # sub-tile width (512)
NBT = N // 128             # batch tiles of 128 rows (32)
TC = 3                     # timesteps per x-chunk (32-partition stride each)
NCH = (T + TC - 1) // TC   # chunks (17: sixteen of 3, one of 2)


def _build_nc(trace_label=None):
    import concourse.bacc as bacc
    import concourse.tile as tile
    from concourse import mybir
    from concourse.masks import make_identity

    F32 = mybir.dt.float32
    F32R = mybir.dt.float32r
    ACTF = mybir.ActivationFunctionType

    nc = bacc.Bacc("TRN2", target_bir_lowering=False, debug=True)

    seq = nc.dram_tensor("seq", [N, T * I], F32, kind="ExternalInput")
    wih_t = nc.dram_tensor("wih_t", [I, 4 * H], F32, kind="ExternalInput")
    whh_t = nc.dram_tensor("whh_t", [H, 4 * H], F32, kind="ExternalInput")
    bias = nc.dram_tensor("bias", [H, 4], F32, kind="ExternalInput")
    h_out = nc.dram_tensor("h_out", [H, N], F32, kind="ExternalOutput")
    c_out = nc.dram_tensor("c_out", [H, N], F32, kind="ExternalOutput")

    with tile.TileContext(nc) as tc, ExitStack() as ctx:
        consts = ctx.enter_context(tc.tile_pool(name="consts", bufs=1))
        state = ctx.enter_context(tc.tile_pool(name="state", bufs=1))
        xchunk = ctx.enter_context(tc.tile_pool(name="xchunk", bufs=2))
        xn_pool = ctx.enter_context(tc.tile_pool(name="xn", bufs=4))
        gates = ctx.enter_context(tc.tile_pool(name="gates", bufs=2))
        tmps = ctx.enter_context(tc.tile_pool(name="tmps", bufs=2))
        psum = ctx.enter_context(tc.tile_pool(name="psum", bufs=6, space="PSUM"))
        tpsum = ctx.enter_context(tc.tile_pool(name="tpsum", bufs=2, space="PSUM"))

        # --- constants ---------------------------------------------------
        ident = consts.tile([128, 128], F32)
        make_identity(nc, ident)

        # W_ih^T replicated at partition bases 0/32/64 so the x-part lhsT
        # base matches the rhs (xT) base for each timestep-in-chunk.
        wih_f = consts.tile([TC * 32, 4 * H], F32)
        whh_f = consts.tile([H, 4 * H], F32)
        bias_sb = consts.tile([H, 4], F32)
        for r in range(TC):
            nc.sync.dma_start(
                out=wih_f[r * 32:r * 32 + I, :], in_=wih_t[:, :]
            )
        nc.sync.dma_start(out=whh_f, in_=whh_t[:, :])
        nc.sync.dma_start(out=bias_sb, in_=bias[:, :])
        # round weights to fp32r once (only rows [32r, 32r+16) are read)
        wih = consts.tile([TC * 32, 4 * H], F32R)
        whh = consts.tile([H, 4 * H], F32R)
        for r in range(TC):
            nc.vector.tensor_copy(
                out=wih[r * 32:r * 32 + I, :], in_=wih_f[r * 32:r * 32 + I, :]
            )
        nc.vector.tensor_copy(out=whh, in_=whh_f)

        # --- persistent state -------------------------------------------
        h_st = [state.tile([H, NS], F32R, tag=f"h{k}", name=f"h{k}") for k in range(NSUB)]
        c_st = [state.tile([H, NS], F32, tag=f"c{k}", name=f"c{k}") for k in range(NSUB)]

        gate_funcs = (ACTF.Sigmoid, ACTF.Sigmoid, ACTF.Tanh, ACTF.Sigmoid)

        import concourse.bass as bass

        for j in range(NCH):
            tsteps = min(TC, T - j * TC)
            ncols = tsteps * 32  # 32 xT partitions per timestep (16 real)
            # transpose this chunk of x into xT: timestep ts of the chunk
            # lands on partitions [32*ts, 32*ts+16) (legal matmul bases),
            # with junk (next timestep's data) in the upper 16.
            xT = xchunk.tile([TC * 32, N], F32R, tag="xT", name=f"xT{j}")
            for b in range(NBT):
                xn = xn_pool.tile([128, TC * 32], F32, tag="xn", name=f"xn{j}_{b}")
                src0 = (j * TC) * I
                if tsteps == TC:
                    # one DMA: 3 overlapping 32-col reads at 16-col stride
                    nc.sync.dma_start(
                        out=xn[:, :ncols].rearrange("p (g c) -> p g c", g=TC),
                        in_=bass.AP(
                            tensor=seq,
                            offset=(b * 128) * (T * I) + src0,
                            ap=[[T * I, 128], [I, TC], [1, 32]],
                        ),
                    )
                else:
                    # tail chunk (t=48,49): stay in-bounds
                    nc.sync.dma_start(
                        out=xn[:, 0:32],
                        in_=seq[b * 128:(b + 1) * 128, src0:src0 + 32],
                    )
                    nc.sync.dma_start(
                        out=xn[:, 32:48],
                        in_=seq[b * 128:(b + 1) * 128, src0 + I:src0 + 2 * I],
                    )
                tp = tpsum.tile([TC * 32, 128], F32, tag="tp", name=f"tp{j}_{b}")
                tncols = ncols if tsteps == TC else 48
                nc.tensor.transpose(tp[:tncols, :], xn[:, :tncols], ident)
                nc.vector.tensor_copy(
                    out=xT[:tncols, b * 128:(b + 1) * 128], in_=tp[:tncols, :]
                )

            for ts in range(tsteps):
                t = j * TC + ts
                xrow = xT[ts * 32:ts * 32 + I, :]
                for k in range(NSUB):
                    ps = [
                        psum.tile([H, NS], F32, tag="ps", name=f"ps{g}_{t}_{k}") for g in range(4)
                    ]
                    for g in range(4):
                        nc.tensor.matmul(
                            ps[g],
                            wih[ts * 32:ts * 32 + I, g * H:(g + 1) * H],
                            xrow[:, k * NS:(k + 1) * NS],
                            start=True,
                            stop=(t == 0),
                        )
                    if t > 0:
                        for g in range(4):
                            nc.tensor.matmul(
                                ps[g],
                                whh[:, g * H:(g + 1) * H],
                                h_st[k],
                                start=False,
                                stop=True,
                            )
                    acts = []
                    for g in range(4):
                        a = gates.tile([H, NS], F32, tag=f"act{g}")
                        nc.scalar.activation(
                            out=a,
                            in_=ps[g],
                            func=gate_funcs[g],
                            bias=bias_sb[:, g:g + 1],
                        )
                        acts.append(a)
                    i_a, f_a, g_a, o_a = acts
                    if t == 0:
                        # c0 = i*g ; h0 = o*tanh(c0)
                        nc.vector.tensor_mul(out=c_st[k], in0=i_a, in1=g_a)
                    else:
                        t1 = tmps.tile([H, NS], F32, tag="t1")
                        nc.vector.tensor_mul(out=t1, in0=i_a, in1=g_a)
                        c2 = tmps.tile([H, NS], F32, tag="c2")
                        nc.gpsimd.tensor_mul(out=c2, in0=f_a, in1=c_st[k])
                        nc.vector.tensor_add(out=c_st[k], in0=c2, in1=t1)
                    tc_t = tmps.tile([H, NS], F32, tag="tc")
                    nc.scalar.activation(
                        out=tc_t, in_=c_st[k], func=ACTF.Tanh
                    )
                    if k < 4:
                        nc.gpsimd.tensor_mul(out=h_st[k], in0=o_a, in1=tc_t)
                    else:
                        nc.vector.tensor_mul(out=h_st[k], in0=o_a, in1=tc_t)

        F32 = F32  # noqa
        for k in range(NSUB):
            nc.sync.dma_start(
                out=h_out[:, k * NS:(k + 1) * NS], in_=h_st[k].bitcast(F32)
            )
            nc.sync.dma_start(
                out=c_out[:, k * NS:(k + 1) * NS], in_=c_st[k]
            )

    nc.finalize()
    return nc


def _numpy_fallback(sequence, mask, W_ih, W_hh, b_ih, b_hh):
    nb, na, nt, _ = sequence.shape
    hdim = W_hh.shape[1]
    h = np.zeros((nb, na, hdim), np.float32)
    c = np.zeros((nb, na, hdim), np.float32)
    bias = (b_ih + b_hh).astype(np.float32)

    def sig(x):
        return 1.0 / (1.0 + np.exp(-x))

    for t in range(nt):
        x = sequence[:, :, t, :]
        gates = x @ W_ih.T + h @ W_hh.T + bias
        i_g = gates[..., 0 * hdim:1 * hdim]
        f_g = gates[..., 1 * hdim:2 * hdim]
        g_g = gates[..., 2 * hdim:3 * hdim]
        o_g = gates[..., 3 * hdim:4 * hdim]
        c_new = sig(f_g) * c + sig(i_g) * np.tanh(g_g)
        h_new = sig(o_g) * np.tanh(c_new)
        m = mask[:, :, t][..., None]
        h = m * h_new + (1.0 - m) * h
        c = m * c_new + (1.0 - m) * c
    return h, c


def kernel(sequence, mask, W_ih, W_hh, b_ih, b_hh):
    sequence = np.asarray(sequence, dtype=np.float32)
    mask = np.asarray(mask, dtype=np.float32)
    W_ih = np.asarray(W_ih, dtype=np.float32)
    W_hh = np.asarray(W_hh, dtype=np.float32)
    b_ih = np.asarray(b_ih, dtype=np.float32)
    b_hh = np.asarray(b_hh, dtype=np.float32)

    if not np.all(mask == 1.0):
        return _numpy_fallback(sequence, mask, W_ih, W_hh, b_ih, b_hh)

    from concourse.bass_utils import run_bass_kernel_spmd

    seq_flat = np.ascontiguousarray(sequence.reshape(B * A, T * I))
    wih_t = np.ascontiguousarray(W_ih.T)              # [I, 4H]
    whh_t = np.ascontiguousarray(W_hh.T)              # [H, 4H]
    bias = np.ascontiguousarray((b_ih + b_hh).reshape(4, H).T)  # [H, 4]

    in_maps = []
    for cidx in range(N_CORES):
        in_maps.append({
            "seq": np.ascontiguousarray(seq_flat[cidx * N:(cidx + 1) * N]),
            "wih_t": wih_t,
            "whh_t": whh_t,
            "bias": bias,
        })

    nc = _build_nc()
    kernel.last_nc = nc
    trace = bool(int(os.environ.get("LSTM_KERNEL_TRACE", "0")))
    res = run_bass_kernel_spmd(
        nc, in_maps, core_ids=list(range(N_CORES)), trace=trace
    )
    if trace and res.exec_time_ns is not None:
        print(f"HW exec time: {res.exec_time_ns} ns")
        kernel.last_exec_time_ns = res.exec_time_ns
        kernel.last_trace = res.instructions_and_trace
    h_full = np.empty((B, A, H), np.float32)
    c_full = np.empty((B, A, H), np.float32)
    for cidx in range(N_CORES):
        hT = res.results[cidx]["h_out"]  # [H, N]
        cT = res.results[cidx]["c_out"]
        h_full[cidx * NB:(cidx + 1) * NB] = hT.T.reshape(NB, A, H)
        c_full[cidx * NB:(cidx + 1) * NB] = cT.T.reshape(NB, A, H)
    return h_full, c_full


# revision 10
# speedup vs baseline: 1.1849x; 1.1849x over previous
"""LSTM-cell scan (masked Encoder) on 8 Trainium2 NeuronCores.

Problem: sequence [B=256, A=128, T=50, I=16] f32 through a single
LSTMCell (H=128) scanned over T; returns final (h, c), each [B, A, H].
The mask input is all-ones (per the problem spec), so the mask blend is
an identity and is skipped on-device (a host-side check falls back to a
numpy implementation if a non-trivial mask ever shows up).

Sharding: data-parallel over B across 8 cores (32 batch rows -> 4096
independent sequences per core). Params replicated.

Per-core kernel layout: hidden dim H=128 lives on SBUF partitions, the
4096 sequences on the free dim, split into 8 sub-tiles of 512 (PSUM bank
width). Per timestep and sub-tile, each gate g gets
    psum_g = W_ih_g^T.T @ x_t  (+)  W_hh_g^T.T @ h
as two accumulating fp32r matmuls, then ACT applies sigmoid/tanh with the
per-partition bias operand, and DVE/Pool do the cell update. x_t arrives
batch-major from DRAM and is transposed on-chip via the PE (8 timesteps
per [128,128] transpose). States h (fp32r, feeds matmul) and c (fp32)
persist in SBUF across all 50 steps; only final h/c are written out.
"""

import os
from contextlib import ExitStack

import numpy as np

N_CORES = 8
B, A, T, I, H = 256, 128, 50, 16, 128
NB = B // N_CORES          # batch rows per core
N = NB * A                 # sequences per core (4096)
NSUB = 8                   # N matmul sub-tiles per core
NS = N // NSUB             # matmul sub-tile width (512, PSUM bank)
NP = 4                     # elementwise pair-tiles per core
PW = N // NP               # pair width (1024, two PSUM banks)
NBT = N // 128             # batch tiles of 128 rows (32)
TC = 3                     # timesteps per x-chunk (32-partition stride each)
NCH = (T + TC - 1) // TC   # chunks (17: sixteen of 3, one of 2)


def _build_nc(trace_label=None):
    import concourse.bacc as bacc
    import concourse.tile as tile
    from concourse import mybir
    from concourse.masks import make_identity

    F32 = mybir.dt.float32
    F32R = mybir.dt.float32r
    ACTF = mybir.ActivationFunctionType

    nc = bacc.Bacc("TRN2", target_bir_lowering=False, debug=True)

    seq = nc.dram_tensor("seq", [N, T * I], F32, kind="ExternalInput")
    wih_t = nc.dram_tensor("wih_t", [I, 4 * H], F32, kind="ExternalInput")
    whh_t = nc.dram_tensor("whh_t", [H, 4 * H], F32, kind="ExternalInput")
    bias = nc.dram_tensor("bias", [H, 4], F32, kind="ExternalInput")
    h_out = nc.dram_tensor("h_out", [H, N], F32, kind="ExternalOutput")
    c_out = nc.dram_tensor("c_out", [H, N], F32, kind="ExternalOutput")

    with tile.TileContext(nc) as tc, ExitStack() as ctx:
        consts = ctx.enter_context(tc.tile_pool(name="consts", bufs=1))
        state = ctx.enter_context(tc.tile_pool(name="state", bufs=1))
        xchunk = ctx.enter_context(tc.tile_pool(name="xchunk", bufs=2))
        xn_pool = ctx.enter_context(tc.tile_pool(name="xn", bufs=4))
        gates = ctx.enter_context(tc.tile_pool(name="gates", bufs=2))
        tmps = ctx.enter_context(tc.tile_pool(name="tmps", bufs=2))
        psum = ctx.enter_context(tc.tile_pool(name="psum", bufs=3, space="PSUM"))
        tpsum = ctx.enter_context(tc.tile_pool(name="tpsum", bufs=2, space="PSUM"))

        # --- constants ---------------------------------------------------
        ident = consts.tile([128, 128], F32)
        make_identity(nc, ident)

        # W_ih^T replicated at partition bases 0/32/64 so the x-part lhsT
        # base matches the rhs (xT) base for each timestep-in-chunk.
        wih_f = consts.tile([TC * 32, 4 * H], F32)
        whh_f = consts.tile([H, 4 * H], F32)
        bias_sb = consts.tile([H, 4], F32)
        for r in range(TC):
            nc.sync.dma_start(
                out=wih_f[r * 32:r * 32 + I, :], in_=wih_t[:, :]
            )
        nc.sync.dma_start(out=whh_f, in_=whh_t[:, :])
        nc.sync.dma_start(out=bias_sb, in_=bias[:, :])
        # round weights to fp32r once (only rows [32r, 32r+16) are read)
        wih = consts.tile([TC * 32, 4 * H], F32R)
        whh = consts.tile([H, 4 * H], F32R)
        for r in range(TC):
            nc.vector.tensor_copy(
                out=wih[r * 32:r * 32 + I, :], in_=wih_f[r * 32:r * 32 + I, :]
            )
        nc.vector.tensor_copy(out=whh, in_=whh_f)

        # --- persistent state -------------------------------------------
        h_st = [state.tile([H, PW], F32R, tag=f"h{k}", name=f"h{k}") for k in range(NP)]
        c_st = [state.tile([H, PW], F32, tag=f"c{k}", name=f"c{k}") for k in range(NP)]

        gate_funcs = (ACTF.Sigmoid, ACTF.Sigmoid, ACTF.Tanh, ACTF.Sigmoid)

        import concourse.bass as bass

        for j in range(NCH):
            tsteps = min(TC, T - j * TC)
            ncols = tsteps * 32  # 32 xT partitions per timestep (16 real)
            # transpose this chunk of x into xT: timestep ts of the chunk
            # lands on partitions [32*ts, 32*ts+16) (legal matmul bases),
            # with junk (next timestep's data) in the upper 16.
            xT = xchunk.tile([TC * 32, N], F32R, tag="xT", name=f"xT{j}")
            for b in range(NBT):
                xn = xn_pool.tile([128, TC * 32], F32, tag="xn", name=f"xn{j}_{b}")
                src0 = (j * TC) * I
                if tsteps == TC:
                    # one DMA: 3 overlapping 32-col reads at 16-col stride
                    nc.sync.dma_start(
                        out=xn[:, :ncols].rearrange("p (g c) -> p g c", g=TC),
                        in_=bass.AP(
                            tensor=seq,
                            offset=(b * 128) * (T * I) + src0,
                            ap=[[T * I, 128], [I, TC], [1, 32]],
                        ),
                    )
                else:
                    # tail chunk (t=48,49): stay in-bounds
                    nc.sync.dma_start(
                        out=xn[:, 0:32],
                        in_=seq[b * 128:(b + 1) * 128, src0:src0 + 32],
                    )
                    nc.sync.dma_start(
                        out=xn[:, 32:48],
                        in_=seq[b * 128:(b + 1) * 128, src0 + I:src0 + 2 * I],
                    )
                tp = tpsum.tile([TC * 32, 128], F32, tag="tp", name=f"tp{j}_{b}")
                tncols = ncols if tsteps == TC else 48
                nc.tensor.transpose(tp[:tncols, :], xn[:, :tncols], ident)
                nc.vector.tensor_copy(
                    out=xT[:tncols, b * 128:(b + 1) * 128], in_=tp[:tncols, :]
                )

            for ts in range(tsteps):
                t = j * TC + ts
                xrow = xT[ts * 32:ts * 32 + I, :]
                wslice = wih[ts * 32:ts * 32 + I, :]
                for k in range(NP):
                    ps = [
                        psum.tile([H, PW], F32, tag="ps", name=f"ps{g}_{t}_{k}")
                        for g in range(4)
                    ]
                    for g in range(4):
                        for half in range(2):
                            lo = half * NS
                            nc.tensor.matmul(
                                ps[g][:, lo:lo + NS],
                                wslice[:, g * H:(g + 1) * H],
                                xrow[:, k * PW + lo:k * PW + lo + NS],
                                start=True,
                                stop=(t == 0),
                            )
                    if t > 0:
                        for g in range(4):
                            for half in range(2):
                                lo = half * NS
                                nc.tensor.matmul(
                                    ps[g][:, lo:lo + NS],
                                    whh[:, g * H:(g + 1) * H],
                                    h_st[k][:, lo:lo + NS],
                                    start=False,
                                    stop=True,
                                )
                    acts = []
                    for g in range(4):
                        a = gates.tile([H, PW], F32, tag=f"act{g}")
                        nc.scalar.activation(
                            out=a,
                            in_=ps[g],
                            func=gate_funcs[g],
                            bias=bias_sb[:, g:g + 1],
                        )
                        acts.append(a)
                    i_a, f_a, g_a, o_a = acts
                    if t == 0:
                        # c0 = i*g ; h0 = o*tanh(c0)
                        nc.vector.tensor_mul(out=c_st[k], in0=i_a, in1=g_a)
                    else:
                        t1 = tmps.tile([H, PW], F32, tag="t1")
                        nc.vector.tensor_mul(out=t1, in0=i_a, in1=g_a)
                        c2 = tmps.tile([H, PW], F32, tag="c2")
                        nc.gpsimd.tensor_mul(out=c2, in0=f_a, in1=c_st[k])
                        nc.vector.tensor_add(out=c_st[k], in0=c2, in1=t1)
                    tc_t = tmps.tile([H, PW], F32, tag="tc")
                    nc.scalar.activation(
                        out=tc_t, in_=c_st[k], func=ACTF.Tanh
                    )
                    if k < 2:
                        nc.gpsimd.tensor_mul(out=h_st[k], in0=o_a, in1=tc_t)
                    else:
                        nc.vector.tensor_mul(out=h_st[k], in0=o_a, in1=tc_t)

        for k in range(NP):
            nc.sync.dma_start(
                out=h_out[:, k * PW:(k + 1) * PW], in_=h_st[k].bitcast(F32)
            )
            nc.sync.dma_start(
                out=c_out[:, k * PW:(k + 1) * PW], in_=c_st[k]
            )

    nc.finalize()
    return nc


def _numpy_fallback(sequence, mask, W_ih, W_hh, b_ih, b_hh):
    nb, na, nt, _ = sequence.shape
    hdim = W_hh.shape[1]
    h = np.zeros((nb, na, hdim), np.float32)
    c = np.zeros((nb, na, hdim), np.float32)
    bias = (b_ih + b_hh).astype(np.float32)

    def sig(x):
        return 1.0 / (1.0 + np.exp(-x))

    for t in range(nt):
        x = sequence[:, :, t, :]
        gates = x @ W_ih.T + h @ W_hh.T + bias
        i_g = gates[..., 0 * hdim:1 * hdim]
        f_g = gates[..., 1 * hdim:2 * hdim]
        g_g = gates[..., 2 * hdim:3 * hdim]
        o_g = gates[..., 3 * hdim:4 * hdim]
        c_new = sig(f_g) * c + sig(i_g) * np.tanh(g_g)
        h_new = sig(o_g) * np.tanh(c_new)
        m = mask[:, :, t][..., None]
        h = m * h_new + (1.0 - m) * h
        c = m * c_new + (1.0 - m) * c
    return h, c


def kernel(sequence, mask, W_ih, W_hh, b_ih, b_hh):
    sequence = np.asarray(sequence, dtype=np.float32)
    mask = np.asarray(mask, dtype=np.float32)
    W_ih = np.asarray(W_ih, dtype=np.float32)
    W_hh = np.asarray(W_hh, dtype=np.float32)
    b_ih = np.asarray(b_ih, dtype=np.float32)
    b_hh = np.asarray(b_hh, dtype=np.float32)

    if not np.all(mask == 1.0):
        return _numpy_fallback(sequence, mask, W_ih, W_hh, b_ih, b_hh)

    from concourse.bass_utils import run_bass_kernel_spmd

    seq_flat = np.ascontiguousarray(sequence.reshape(B * A, T * I))
    wih_t = np.ascontiguousarray(W_ih.T)              # [I, 4H]
    whh_t = np.ascontiguousarray(W_hh.T)              # [H, 4H]
    bias = np.ascontiguousarray((b_ih + b_hh).reshape(4, H).T)  # [H, 4]

    in_maps = []
    for cidx in range(N_CORES):
        in_maps.append({
            "seq": np.ascontiguousarray(seq_flat[cidx * N:(cidx + 1) * N]),
            "wih_t": wih_t,
            "whh_t": whh_t,
            "bias": bias,
        })

    nc = _build_nc()
    kernel.last_nc = nc
    trace = bool(int(os.environ.get("LSTM_KERNEL_TRACE", "0")))
    res = run_bass_kernel_spmd(
        nc, in_maps, core_ids=list(range(N_CORES)), trace=trace
    )
    if trace and res.exec_time_ns is not None:
        print(f"HW exec time: {res.exec_time_ns} ns")
        kernel.last_exec_time_ns = res.exec_time_ns
        kernel.last_trace = res.instructions_and_trace
    h_full = np.empty((B, A, H), np.float32)
    c_full = np.empty((B, A, H), np.float32)
    for cidx in range(N_CORES):
        hT = res.results[cidx]["h_out"]  # [H, N]
        cT = res.results[cidx]["c_out"]
        h_full[cidx * NB:(cidx + 1) * NB] = hT.T.reshape(NB, A, H)
        c_full[cidx * NB:(cidx + 1) * NB] = cT.T.reshape(NB, A, H)
    return h_full, c_full
